# revision 16
# baseline (speedup 1.0000x reference)
"""Trainium2 Bass kernel for nn_CustomSelfAttention_24257975288159.

Reference computation (B=4, L=2048, D=1024, H=16, HD=64, fp32):
  q = x @ Wq + bq ; q[:, 1, :] = cross_cls_sent @ Wq + bq
  k = x @ Wk + bk ; v = x @ Wv + bv
  out = softmax(q k^T / sqrt(HD) + mask) v       (per head)

Sharding: 8 cores = batch (4) x head-group (2). Core c handles batch
c//2 and heads 8*(c%2)..8*(c%2)+7, i.e. columns 512*(c%2)..+512 of the
model dim; QKV weights are column-sharded per head group.

All matmuls run in bf16 (inputs rounded on host / by DVE) with fp32
PSUM accumulation; measured end-to-end rel err ~2e-3 vs the fp32
reference (gate is 2e-2). fp32r was 4x slower on HW (quarter-rate PE).

Division of labor vs the old fp32r kernel:
  - Host pre-transposes x and casts x/W to bf16, so the device loads
    xT [d-part, l-free] straight from HBM (no PE transpose phase).
  - Device emits unnormalized ctx^T [dh-part, lq-free] (fp16) plus
    per-head softmax denominators (fp16); host divides and transposes.
    This removes the on-device normalize/transpose epilogue.

Per-core device algorithm:
  1. Projections: qT,kT in [head-dim-part, l-free] layout (W chunk
     stationary, xT moving); v in [l-part, dh-free] layout with a ones
     column appended per head so the ctx matmul also produces softmax
     denominators (M=65).
  2. Attention per head-pair (p), lq quarter (q), lk chunk (c):
     transposed scores sT[lk-part, lq-free] via a row-tiled matmul pair
     (K=64 in each half of the PE array, both heads into one [128,1024]
     psum tile), one fused exp(SCALE*s + mask) on ScalarE covering both
     heads -> pt (bf16), then two M=65 ctx matmuls accumulating ctx^T
     plus the denominator row in PSUM.
  3. The exp stream on ScalarE is the bottleneck (~1.15us per step,
     256 steps); scores run LOOKAHEAD steps ahead and projection
     windows for the next pair are emitted one-per-few-steps between
     attention steps so ScalarE never waits at pair boundaries.
"""
import numpy as np
import ml_dtypes

import concourse.bass as bass
import concourse.mybir as mybir
import concourse.tile as tile

F32 = mybir.dt.float32
BF16 = mybir.dt.bfloat16
F16 = mybir.dt.float16

B, L, D, H = 4, 2048, 1024, 16
HD = D // H          # 64
SCALE = float(1.0 / np.sqrt(HD))
DG = D // 2          # 512 output columns per core (8 heads)
NCORES = 8
LC = L // 128        # 16 lk-chunks
DC = D // 128        # 8 d-chunks
GC = DG // 128       # 4 dh-chunks per core = head pairs
HS = HD + 1          # 65: v columns per head incl ones column

_CACHED = {}


# ---------------------------------------------------------------------------
# walrus workaround: this build rejects >1 sync-wait per instruction.
# Spill excess waits onto single-wait NOPs on the same engine.
# ---------------------------------------------------------------------------
def _split_excess_waits(nc, max_waits=1):
    counter = 0
    for fn in nc.m.functions:
        for blk in fn.blocks:
            il = blk.instructions
            out = []
            changed = False
            for ins in il:
                si = getattr(ins, "sync_info", None)
                waits = list(si.on_wait) if si is not None and si.on_wait else []
                if len(waits) > max_waits:
                    si.on_wait = waits[:max_waits]
                    for w in waits[max_waits:]:
                        counter += 1
                        out.append(
                            mybir.InstNoOp(
                                name=f"waitsplit_{counter}",
                                engine=ins.engine,
                                bass_nofuse=True,
                                sync_info=mybir.SyncInfo(on_wait=[w], on_update=[]),
                            )
                        )
                    changed = True
                out.append(ins)
            if changed:
                il.clear()
                il.extend(out)
    return counter


def _build_program(repeat=1, hw_loop=0):
    nc = bass.Bass()

    xt_d = nc.declare_dram_parameter("xt", [D, L], BF16, isOutput=False)
    wq_d = nc.declare_dram_parameter("wq", [D, DG], BF16, isOutput=False)
    wk_d = nc.declare_dram_parameter("wk", [D, DG], BF16, isOutput=False)
    wv_d = nc.declare_dram_parameter("wv", [D, DG], BF16, isOutput=False)
    qc_d = nc.declare_dram_parameter("qcross", [128, GC], F32, isOutput=False)
    bq_d = nc.declare_dram_parameter("bq", [128, GC], F32, isOutput=False)
    bk_d = nc.declare_dram_parameter("bk", [128, GC], F32, isOutput=False)
    bv_d = nc.declare_dram_parameter("bv", [1, DG], F32, isOutput=False)
    mk_d = nc.declare_dram_parameter("maskm", [128, LC], F32, isOutput=False)
    out_d = nc.declare_dram_parameter("out", [DG, L], F16, isOutput=True)
    den_d = nc.declare_dram_parameter("den", [2 * GC, L], F16, isOutput=True)

    with tile.TileContext(nc, pool_alloc_mode="queue") as tc:
        with (
            tc.tile_pool(name="const", bufs=1) as const,
            tc.tile_pool(name="qkv", bufs=1) as qkv,
            tc.tile_pool(name="xt", bufs=1) as xtp,
            tc.tile_pool(name="wqk", bufs=1) as wqkp,
            tc.tile_pool(name="pt", bufs=3) as ptp,
            tc.tile_pool(name="fin", bufs=1) as finp,
        ):
            # ---- constants ----
            ones1 = const.tile([1, 128], F32)
            nc.vector.memset(ones1[:], 1.0)
            ones8 = const.tile([128, 8], BF16)
            nc.vector.memset(ones8[:], 1.0)
            qc_sb = const.tile([128, GC], F32)
            nc.sync.dma_start(out=qc_sb[:], in_=qc_d[:, :])
            bq_sb = const.tile([128, GC], F32)
            nc.sync.dma_start(out=bq_sb[:], in_=bq_d[:, :])
            bk_sb = const.tile([128, GC], F32)
            nc.sync.dma_start(out=bk_sb[:], in_=bk_d[:, :])
            bv_sb = const.tile([1, DG], F32)
            nc.sync.dma_start(out=bv_sb[:], in_=bv_d[:, :])
            mk_sb = const.tile([128, LC], F32)
            nc.sync.dma_start(out=mk_sb[:], in_=mk_d[:, :])
            bias_v = const.tile([128, DG], F32)

            def body():
                vt = [
                    qkv.tile([128, 8 * HS], BF16, tag=f"v{lc}", bufs=2,
                             name=f"v{lc}")
                    for lc in range(LC)
                ]
                xT = [
                    xtp.tile([128, L], BF16, tag=f"xT{dc}", bufs=2,
                             name=f"xT{dc}")
                    for dc in range(DC)
                ]
                ctxT = [
                    finp.tile([128, L], F16, tag=f"ctxT{p}", name=f"ctxT{p}")
                    for p in range(GC)
                ]
                # denominator rows: DVE output partition bases must be
                # 32-aligned, so pair p's two head rows live at partition
                # 32*p of two separate tiles (head A / head B).
                den_a = finp.tile([97, L], F16, tag="denA", bufs=2)
                den_b = finp.tile([97, L], F16, tag="denB", bufs=2)

                # ---- load xT [d-part, l-free] straight from HBM ----
                # Each dynamic HWDGE ring moves only ~50GB/s (measured; the
                # SWDGE path is unavailable — DynamicDMA disabled in this
                # walrus pipeline), so the 4MB x load alternates between the
                # ACT and SP rings to double throughput. Fewer, bigger DMAs
                # win: a finer 16-way split measured strictly worse.
                for dc in range(DC):
                    eng = nc.scalar if dc % 2 == 0 else nc.sync
                    eng.dma_start(
                        out=xT[dc][:], in_=xt_d[dc * 128:(dc + 1) * 128, :]
                    )

                with tc.tile_pool(name="psBC", bufs=1, space="PSUM") as psBC:
                    with nc.named_scope("biasv"):
                        psb = psBC.tile([128, 512], F32, tag="proj", bufs=2)
                        nc.tensor.matmul(
                            psb[:], ones1[:], bv_sb[:], start=True, stop=True
                        )
                        nc.vector.tensor_copy(bias_v[:], psb[:])

                    # ---- projection work units ----
                    qk = [[None, None] for _ in range(GC)]  # [p][0]=qT, [1]=kT
                    wqk_sb = {}

                    def proj_w_dma(p, qi):
                        # DMA the 8 weight chunks for (pair p, q/k) to SBUF
                        wd = wq_d if qi == 0 else wk_d
                        wts = []
                        for dc in range(DC):
                            wt = wqkp.tile(
                                [128, 128], BF16, tag="wqk", bufs=32,
                                name=f"w_{p}_{qi}_{dc}",
                            )
                            nc.scalar.dma_start(
                                out=wt[:],
                                in_=wd[dc * 128:(dc + 1) * 128,
                                       p * 128:(p + 1) * 128],
                            )
                            wts.append(wt)
                        wqk_sb[(p, qi)] = wts

                    def proj_window(p, qi, w):
                        # one lq window (512) of the qT/kT projection for pair p
                        tagname = "qTs" if qi == 0 else "kTs"
                        if w == 0:
                            qk[p][qi] = qkv.tile(
                                [128, L], BF16, tag=tagname, bufs=2,
                                name=f"{tagname}{p}",
                            )
                        dst = qk[p][qi]
                        bias_sb = bq_sb if qi == 0 else bk_sb
                        wts = wqk_sb[(p, qi)]
                        with nc.named_scope(f"proj{p}_{qi}_{w}"):
                            psw = psBC.tile([128, 512], F32, tag="proj", bufs=2)
                            for dc in range(DC):
                                nc.tensor.matmul(
                                    psw[:],
                                    wts[dc][:],
                                    xT[dc][:, w * 512:(w + 1) * 512],
                                    start=(dc == 0),
                                    stop=(dc == DC - 1),
                                )
                            nc.vector.tensor_scalar_add(
                                dst[:, w * 512:(w + 1) * 512],
                                psw[:],
                                bias_sb[:, p:p + 1],
                            )
                            if qi == 0 and w == 0:
                                # q row-1 fix (CLS_sent cross query)
                                nc.vector.tensor_copy(
                                    dst[:, 1:2], qc_sb[:, p:p + 1]
                                )

                    wv_sb = []

                    def proj_v_dma():
                        for dc in range(DC):
                            wt = wqkp.tile(
                                [128, DG], BF16, tag="wv", bufs=8,
                                name=f"wv{dc}",
                            )
                            nc.scalar.dma_start(
                                out=wt[:], in_=wv_d[dc * 128:(dc + 1) * 128, :]
                            )
                            wv_sb.append(wt)

                    def proj_v_unit(lc):
                        # v projection for one lk chunk, with ones columns
                        with nc.named_scope(f"projv{lc}"):
                            psv = psBC.tile([128, 512], F32, tag="proj", bufs=2)
                            for dc in range(DC):
                                nc.tensor.matmul(
                                    psv[:],
                                    xT[dc][:, lc * 128:(lc + 1) * 128],
                                    wv_sb[dc][:],
                                    start=(dc == 0),
                                    stop=(dc == DC - 1),
                                )
                            v = vt[lc]
                            ones_cols = v.rearrange(
                                "p (h s) -> p h s", s=HS
                            )[:, :, HD]
                            nc.vector.tensor_copy(ones_cols, ones8[:])
                            for h in range(8):
                                nc.vector.tensor_add(
                                    v[:, h * HS:h * HS + HD],
                                    psv[:, h * HD:(h + 1) * HD],
                                    bias_v[:, h * HD:(h + 1) * HD],
                                )

                    def scores(p, q, c):
                        q_t, k_t = qk[p][0], qk[p][1]
                        lq = q * 512
                        sAB = psBC.tile(
                            [128, 1024], F32, tag="sAB", bufs=2,
                            name=f"sAB_{p}_{q}_{c}",
                        )
                        nc.tensor.matmul(
                            sAB[:, 0:512],
                            k_t[0:64, c * 128:(c + 1) * 128],
                            q_t[0:64, lq:lq + 512],
                            start=True, stop=True,
                            tile_position=(0, 0),
                        )
                        nc.tensor.matmul(
                            sAB[:, 512:1024],
                            k_t[64:128, c * 128:(c + 1) * 128],
                            q_t[64:128, lq:lq + 512],
                            start=True, stop=True,
                            tile_position=(64, 0),
                        )
                        return sAB

                    # ---- emission schedule ----
                    # step s = (pair p, lq quarter q, lk chunk c)
                    NSTEP = GC * 4 * LC
                    LOOKAHEAD = 2

                    def step_pqc(s_):
                        p_, r = divmod(s_, 4 * LC)
                        q_, c_ = divmod(r, LC)
                        return p_, q_, c_

                    # Extra (non-attention) work units, emitted between steps
                    # so the PE interleaves them without starving ScalarE.
                    # extra[s] runs just before scores(s+LOOKAHEAD)/exp(s).
                    extra = [[] for _ in range(NSTEP)]
                    # v units: needed by ctx at step c (pair 0); keep 3 ahead.
                    for lc in range(4, LC):
                        extra[lc - 3].append(("v", lc))
                    # pair 1..3 q/k projections: spread over the previous
                    # pair's steps, one window every 5 steps, starting after
                    # pair 0's own windows are all emitted (step 16) so the
                    # weight-slot reuse can never cycle with the PE stream.
                    for p in range(1, GC):
                        base = (p - 1) * 4 * LC + 16
                        for j in range(8):      # 4 q windows then 4 k windows
                            qi, w = (0, j) if j < 4 else (1, j - 4)
                            extra[base + 5 * j].append(("w", p, qi, w))
                        extra[base].insert(0, ("wdma", p))

                    # prologue: weight DMAs + just enough projection to start
                    proj_w_dma(0, 0)
                    proj_w_dma(0, 1)
                    proj_v_dma()
                    proj_window(0, 0, 0)
                    proj_window(0, 1, 0)
                    for lc in range(4):
                        proj_v_unit(lc)
                    # remaining pair-0 windows, emitted before the steps that
                    # need them (k window j -> step 4j; q window w -> step 16w)
                    extra[1].append(("w", 0, 1, 1))
                    extra[3].append(("w", 0, 0, 1))
                    extra[5].append(("w", 0, 1, 2))
                    extra[9].append(("w", 0, 1, 3))
                    extra[11].append(("w", 0, 0, 2))
                    extra[13].append(("w", 0, 0, 3))

                    def emit_extra(u):
                        if u[0] == "v":
                            proj_v_unit(u[1])
                        elif u[0] == "w":
                            proj_window(u[1], u[2], u[3])
                        elif u[0] == "wdma":
                            proj_w_dma(u[1], 0)
                            proj_w_dma(u[1], 1)

                    sABs = {}
                    for s_ in range(LOOKAHEAD):
                        sABs[s_] = scores(*step_pqc(s_))
                    cA = cB = None
                    for s_ in range(NSTEP):
                        p, q, c = step_pqc(s_)
                        for u in extra[s_]:
                            emit_extra(u)
                        if c == 0:
                            cA = psBC.tile([65, 512], F32, tag="ctxA",
                                           name=f"cA{p}_{q}")
                            cB = psBC.tile([65, 512], F32, tag="ctxB",
                                           name=f"cB{p}_{q}")
                        if s_ + LOOKAHEAD < NSTEP:
                            sABs[s_ + LOOKAHEAD] = scores(*step_pqc(s_ + LOOKAHEAD))
                        sAB = sABs.pop(s_)
                        pt = ptp.tile([128, 1024], BF16, tag="pt")
                        nc.scalar.activation(
                            pt[:],
                            sAB[:],
                            mybir.ActivationFunctionType.Exp,
                            bias=mk_sb[:, c:c + 1],
                            scale=SCALE,
                        )
                        hA = 2 * p * HS
                        hB = (2 * p + 1) * HS
                        nc.tensor.matmul(
                            cA[:],
                            vt[c][:, hA:hA + HS],
                            pt[:, 0:512],
                            start=(c == 0), stop=(c == LC - 1),
                        )
                        nc.tensor.matmul(
                            cB[:],
                            vt[c][:, hB:hB + HS],
                            pt[:, 512:1024],
                            start=(c == 0), stop=(c == LC - 1),
                        )
                        if c == LC - 1:
                            lq = q * 512
                            nc.vector.tensor_copy(
                                ctxT[p][0:64, lq:lq + 512], cA[0:64, :]
                            )
                            nc.vector.tensor_copy(
                                ctxT[p][64:128, lq:lq + 512], cB[0:64, :]
                            )
                            nc.vector.tensor_copy(
                                den_a[32 * p:32 * p + 1, lq:lq + 512],
                                cA[64:65, :],
                            )
                            nc.vector.tensor_copy(
                                den_b[32 * p:32 * p + 1, lq:lq + 512],
                                cB[64:65, :],
                            )
                            if q == 3:
                                nc.sync.dma_start(
                                    out=out_d[p * 128:(p + 1) * 128, :],
                                    in_=ctxT[p][:],
                                )
                    for p in range(GC):
                        nc.sync.dma_start(
                            out=den_d[2 * p:2 * p + 1, :],
                            in_=den_a[32 * p:32 * p + 1, :],
                        )
                        nc.sync.dma_start(
                            out=den_d[2 * p + 1:2 * p + 2, :],
                            in_=den_b[32 * p:32 * p + 1, :],
                        )

            if hw_loop:
                with tc.For_i(0, hw_loop, 1):
                    body()
            else:
                for _rep in range(repeat):
                    body()

    _split_excess_waits(nc)
    return nc


def _prep_in_maps(x, attn_mask, cross_cls_sent, Wq, bq, Wk, bk, Wv, bv):
    bf = ml_dtypes.bfloat16
    x = np.asarray(x, dtype=np.float32)
    attn_mask = np.asarray(attn_mask, dtype=np.float32)
    cross = np.asarray(cross_cls_sent, dtype=np.float32)
    Wq = np.asarray(Wq, dtype=np.float32)
    bq = np.asarray(bq, dtype=np.float32)
    Wk = np.asarray(Wk, dtype=np.float32)
    bk = np.asarray(bk, dtype=np.float32)
    Wv = np.asarray(Wv, dtype=np.float32)
    bv = np.asarray(bv, dtype=np.float32)

    xt_bf = [np.ascontiguousarray(x[b].T.astype(bf)) for b in range(B)]
    w_bf = {}
    for g in range(2):
        cols = slice(g * DG, (g + 1) * DG)
        w_bf[g] = (
            np.ascontiguousarray(Wq[:, cols].astype(bf)),
            np.ascontiguousarray(Wk[:, cols].astype(bf)),
            np.ascontiguousarray(Wv[:, cols].astype(bf)),
        )

    in_maps = []
    for c in range(NCORES):
        b = c // 2
        g = c % 2
        cols = slice(g * DG, (g + 1) * DG)
        qcross = cross[b] @ Wq[:, cols] + bq[cols]  # (512,) host matvec
        in_maps.append(
            {
                "xt": xt_bf[b],
                "wq": w_bf[g][0],
                "wk": w_bf[g][1],
                "wv": w_bf[g][2],
                "qcross": np.ascontiguousarray(
                    qcross.reshape(GC, 128).T.astype(np.float32)
                ),
                "bq": np.ascontiguousarray(bq[cols].reshape(GC, 128).T),
                "bk": np.ascontiguousarray(bk[cols].reshape(GC, 128).T),
                "bv": np.ascontiguousarray(bv[cols].reshape(1, DG)),
                "maskm": np.ascontiguousarray(
                    attn_mask[b, 0, 0].reshape(LC, 128).T
                ),
            }
        )
    return in_maps


def kernel(x, attn_mask, cross_cls_sent, Wq, bq, Wk, bk, Wv, bv):
    from concourse.bass_utils import run_bass_kernel_spmd

    if "nc" not in _CACHED:
        _CACHED["nc"] = _build_program()
    nc = _CACHED["nc"]

    in_maps = _prep_in_maps(
        x, attn_mask, cross_cls_sent, Wq, bq, Wk, bk, Wv, bv
    )
    res = run_bass_kernel_spmd(nc, in_maps, list(range(NCORES)))
    out = np.empty((B, L, D), dtype=np.float32)
    for c in range(NCORES):
        b = c // 2
        g = c % 2
        ct = res.results[c]["out"].astype(np.float32)     # [DG, L] unnormalized
        dn = res.results[c]["den"].astype(np.float32)     # [8, L]
        out[b][:, g * DG:(g + 1) * DG] = (ct / np.repeat(dn, HD, axis=0)).T
    return out


# revision 27
# speedup vs baseline: 1.0887x; 1.0887x over previous
"""Trainium2 Bass kernel for nn_CustomSelfAttention_24257975288159.

Reference computation (B=4, L=2048, D=1024, H=16, HD=64, fp32):
  q = x @ Wq + bq ; q[:, 1, :] = cross_cls_sent @ Wq + bq
  k = x @ Wk + bk ; v = x @ Wv + bv
  out = softmax(q k^T / sqrt(HD) + mask) v       (per head)

Sharding: 8 cores = batch (4) x head-group (2). Core c handles batch
c//2 and heads 8*(c%2)..8*(c%2)+7, i.e. columns 512*(c%2)..+512 of the
model dim; QKV weights are column-sharded per head group.

All matmuls run in bf16 (inputs rounded on host / by DVE) with fp32
PSUM accumulation; measured end-to-end rel err ~2e-3 vs the fp32
reference (gate is 2e-2). fp32r was 4x slower on HW (quarter-rate PE).

Division of labor vs the old fp32r kernel:
  - Host pre-transposes x and casts x/W to bf16, so the device loads
    xT [d-part, l-free] straight from HBM (no PE transpose phase).
  - Device emits unnormalized ctx^T [dh-part, lq-free] (fp16) plus
    per-head softmax denominators (fp16); host divides and transposes.
    This removes the on-device normalize/transpose epilogue.

Per-core device algorithm:
  1. Projections: qT,kT in [head-dim-part, l-free] layout (W chunk
     stationary, xT moving); v in [l-part, dh-free] layout with a ones
     column appended per head so the ctx matmul also produces softmax
     denominators (M=65).
  2. Attention per head-pair (p), lq quarter (q), lk chunk (c):
     transposed scores sT[lk-part, lq-free] via a row-tiled matmul pair
     (K=64 in each half of the PE array, both heads into one [128,1024]
     psum tile), one fused exp(SCALE*s + mask) on ScalarE covering both
     heads -> pt (bf16), then two M=65 ctx matmuls accumulating ctx^T
     plus the denominator row in PSUM.
  3. The exp stream on ScalarE is the bottleneck (~1.15us per step,
     256 steps); scores run LOOKAHEAD steps ahead and projection
     windows for the next pair are emitted one-per-few-steps between
     attention steps so ScalarE never waits at pair boundaries.
"""
import numpy as np
import ml_dtypes

import concourse.bass as bass
import concourse.mybir as mybir
import concourse.tile as tile

F32 = mybir.dt.float32
BF16 = mybir.dt.bfloat16
F16 = mybir.dt.float16

B, L, D, H = 4, 2048, 1024, 16
HD = D // H          # 64
SCALE = float(1.0 / np.sqrt(HD))
DG = D // 2          # 512 output columns per core (8 heads)
NCORES = 8
LC = L // 128        # 16 lk-chunks
DC = D // 128        # 8 d-chunks
GC = DG // 128       # 4 dh-chunks per core = head pairs
HS = HD + 1          # 65: v columns per head incl ones column

_CACHED = {}


# ---------------------------------------------------------------------------
# walrus workaround: this build rejects >1 sync-wait per instruction.
# Spill excess waits onto single-wait NOPs on the same engine.
# ---------------------------------------------------------------------------
def _split_excess_waits(nc, max_waits=1):
    counter = 0
    for fn in nc.m.functions:
        for blk in fn.blocks:
            il = blk.instructions
            out = []
            changed = False
            for ins in il:
                si = getattr(ins, "sync_info", None)
                waits = list(si.on_wait) if si is not None and si.on_wait else []
                if len(waits) > max_waits:
                    si.on_wait = waits[:max_waits]
                    for w in waits[max_waits:]:
                        counter += 1
                        out.append(
                            mybir.InstNoOp(
                                name=f"waitsplit_{counter}",
                                engine=ins.engine,
                                bass_nofuse=True,
                                sync_info=mybir.SyncInfo(on_wait=[w], on_update=[]),
                            )
                        )
                    changed = True
                out.append(ins)
            if changed:
                il.clear()
                il.extend(out)
    return counter


def _build_program(repeat=1, hw_loop=0):
    nc = bass.Bass()

    xt_d = nc.declare_dram_parameter("xt", [128, DC * L], BF16, isOutput=False)
    wq_d = nc.declare_dram_parameter("wq", [D, DG], BF16, isOutput=False)
    wk_d = nc.declare_dram_parameter("wk", [D, DG], BF16, isOutput=False)
    wv_d = nc.declare_dram_parameter("wv", [D, DG], BF16, isOutput=False)
    qc_d = nc.declare_dram_parameter("qcross", [128, GC], F32, isOutput=False)
    bq_d = nc.declare_dram_parameter("bq", [128, GC], F32, isOutput=False)
    bk_d = nc.declare_dram_parameter("bk", [128, GC], F32, isOutput=False)
    bv_d = nc.declare_dram_parameter("bv", [1, DG], F32, isOutput=False)
    mk_d = nc.declare_dram_parameter("maskm", [128, LC], F32, isOutput=False)
    out_d = nc.declare_dram_parameter("out", [DG, L], F16, isOutput=True)
    den_d = nc.declare_dram_parameter("den", [2 * GC, L], F16, isOutput=True)

    with tile.TileContext(nc, pool_alloc_mode="queue") as tc:
        with (
            tc.tile_pool(name="const", bufs=1) as const,
            tc.tile_pool(name="qkv", bufs=1) as qkv,
            tc.tile_pool(name="xt", bufs=1) as xtp,
            tc.tile_pool(name="wqk", bufs=1) as wqkp,
            tc.tile_pool(name="pt", bufs=3) as ptp,
            tc.tile_pool(name="fin", bufs=1) as finp,
        ):
            # ---- constants ----
            ones1 = const.tile([1, 128], F32)
            nc.vector.memset(ones1[:], 1.0)
            ones8 = const.tile([128, 8], BF16)
            nc.vector.memset(ones8[:], 1.0)
            qc_sb = const.tile([128, GC], F32)
            nc.sync.dma_start(out=qc_sb[:], in_=qc_d[:, :])
            bq_sb = const.tile([128, GC], F32)
            nc.sync.dma_start(out=bq_sb[:], in_=bq_d[:, :])
            bk_sb = const.tile([128, GC], F32)
            nc.sync.dma_start(out=bk_sb[:], in_=bk_d[:, :])
            bv_sb = const.tile([1, DG], F32)
            nc.sync.dma_start(out=bv_sb[:], in_=bv_d[:, :])
            mk_sb = const.tile([128, LC], F32)
            nc.sync.dma_start(out=mk_sb[:], in_=mk_d[:, :])
            bias_v = const.tile([128, DG], F32)

            def body():
                vt = [
                    qkv.tile([128, 8 * HS], BF16, tag=f"v{lc}", bufs=2,
                             name=f"v{lc}")
                    for lc in range(LC)
                ]
                xT_all = xtp.tile([128, DC * L], BF16, tag="xTall", bufs=2,
                                  name="xTall")
                xT = [xT_all[:, dc * L:(dc + 1) * L] for dc in range(DC)]
                ctxT = [
                    finp.tile([128, L], F16, tag=f"ctxT{p}", name=f"ctxT{p}")
                    for p in range(GC)
                ]
                # denominator rows: DVE output partition bases must be
                # 32-aligned, so pair p's two head rows live at partition
                # 32*p of two separate tiles (head A / head B). 128-partition
                # tiles so the final DMA can use a stride-32 rearrange view.
                den_a = finp.tile([128, L], F16, tag="denA", bufs=2)
                den_b = finp.tile([128, L], F16, tag="denB", bufs=2)

                # ---- load xT [d-part, l-free] straight from HBM ----
                # DMA instruction count is the scarce resource here: each
                # DMA costs ~2us of ring time regardless of size (HW), so
                # inputs are packed host-side and loaded with ONE DMA on the
                # ACT HWDGE ring. Outputs use the SP ring so neither ring
                # carries both directions (head-of-line blocking measured
                # +80us). SWDGE/gpsimd is unavailable (DynamicDMA disabled).
                nc.scalar.dma_start(out=xT_all[:], in_=xt_d[:, :])

                with tc.tile_pool(name="psBC", bufs=1, space="PSUM") as psBC:
                    with nc.named_scope("biasv"):
                        psb = psBC.tile([128, 512], F32, tag="proj", bufs=2)
                        nc.tensor.matmul(
                            psb[:], ones1[:], bv_sb[:], start=True, stop=True
                        )
                        nc.vector.tensor_copy(bias_v[:], psb[:])

                    # ---- projection work units ----
                    qk = [[None, None] for _ in range(GC)]  # [p][0]=qT, [1]=kT
                    wqk_sb = {}

                    def proj_w_dma(p, qi):
                        # one DMA for all 8 d-chunks of (pair p, q/k):
                        # wt[:, dc*128:(dc+1)*128] = wd[dc*128+part, p cols]
                        wd = wq_d if qi == 0 else wk_d
                        wt = wqkp.tile(
                            [128, DC * 128], BF16, tag="wqk", bufs=4,
                            name=f"w_{p}_{qi}",
                        )
                        nc.scalar.dma_start(
                            out=wt[:].rearrange("p (dc j) -> p dc j", j=128),
                            in_=wd.rearrange("(dc p) n -> p dc n", p=128)[
                                :, :, p * 128:(p + 1) * 128
                            ],
                        )
                        wqk_sb[(p, qi)] = [
                            wt[:, dc * 128:(dc + 1) * 128] for dc in range(DC)
                        ]

                    def proj_window(p, qi, w):
                        # one lq window (512) of the qT/kT projection for pair p
                        tagname = "qTs" if qi == 0 else "kTs"
                        if w == 0:
                            qk[p][qi] = qkv.tile(
                                [128, L], BF16, tag=tagname, bufs=2,
                                name=f"{tagname}{p}",
                            )
                        dst = qk[p][qi]
                        bias_sb = bq_sb if qi == 0 else bk_sb
                        wts = wqk_sb[(p, qi)]
                        with nc.named_scope(f"proj{p}_{qi}_{w}"):
                            psw = psBC.tile([128, 512], F32, tag="proj", bufs=2)
                            for dc in range(DC):
                                nc.tensor.matmul(
                                    psw[:],
                                    wts[dc],
                                    xT[dc][:, w * 512:(w + 1) * 512],
                                    start=(dc == 0),
                                    stop=(dc == DC - 1),
                                )
                            nc.vector.tensor_scalar_add(
                                dst[:, w * 512:(w + 1) * 512],
                                psw[:],
                                bias_sb[:, p:p + 1],
                            )
                            if qi == 0 and w == 0:
                                # q row-1 fix (CLS_sent cross query)
                                nc.vector.tensor_copy(
                                    dst[:, 1:2], qc_sb[:, p:p + 1]
                                )

                    wv_sb = []

                    def proj_v_dma():
                        wt = wqkp.tile(
                            [128, DC * DG], BF16, tag="wv", bufs=2, name="wv",
                        )
                        nc.scalar.dma_start(
                            out=wt[:].rearrange("p (dc j) -> p dc j", j=DG),
                            in_=wv_d.rearrange("(dc p) n -> p dc n", p=128),
                        )
                        for dc in range(DC):
                            wv_sb.append(wt[:, dc * DG:(dc + 1) * DG])

                    def proj_v_unit(lc):
                        # v projection for one lk chunk, with ones columns
                        with nc.named_scope(f"projv{lc}"):
                            psv = psBC.tile([128, 512], F32, tag="proj", bufs=2)
                            for dc in range(DC):
                                nc.tensor.matmul(
                                    psv[:],
                                    xT[dc][:, lc * 128:(lc + 1) * 128],
                                    wv_sb[dc],
                                    start=(dc == 0),
                                    stop=(dc == DC - 1),
                                )
                            v = vt[lc]
                            ones_cols = v.rearrange(
                                "p (h s) -> p h s", s=HS
                            )[:, :, HD]
                            nc.vector.tensor_copy(ones_cols, ones8[:])
                            for h in range(8):
                                nc.vector.tensor_add(
                                    v[:, h * HS:h * HS + HD],
                                    psv[:, h * HD:(h + 1) * HD],
                                    bias_v[:, h * HD:(h + 1) * HD],
                                )

                    def scores(p, q, c):
                        q_t, k_t = qk[p][0], qk[p][1]
                        lq = q * 512
                        sAB = psBC.tile(
                            [128, 1024], F32, tag="sAB", bufs=2,
                            name=f"sAB_{p}_{q}_{c}",
                        )
                        nc.tensor.matmul(
                            sAB[:, 0:512],
                            k_t[0:64, c * 128:(c + 1) * 128],
                            q_t[0:64, lq:lq + 512],
                            start=True, stop=True,
                            tile_position=(0, 0),
                        )
                        nc.tensor.matmul(
                            sAB[:, 512:1024],
                            k_t[64:128, c * 128:(c + 1) * 128],
                            q_t[64:128, lq:lq + 512],
                            start=True, stop=True,
                            tile_position=(64, 0),
                        )
                        return sAB

                    # ---- emission schedule ----
                    # step s = (pair p, lq quarter q, lk chunk c)
                    NSTEP = GC * 4 * LC
                    LOOKAHEAD = 2

                    def step_pqc(s_):
                        p_, r = divmod(s_, 4 * LC)
                        q_, c_ = divmod(r, LC)
                        return p_, q_, c_

                    # Extra (non-attention) work units, emitted between steps
                    # so the PE interleaves them without starving ScalarE.
                    # extra[s] runs just before scores(s+LOOKAHEAD)/exp(s).
                    extra = [[] for _ in range(NSTEP)]
                    # v units: needed by ctx at step c (pair 0); keep 3 ahead.
                    for lc in range(4, LC):
                        extra[lc - 3].append(("v", lc))
                    # pair 1..3 q/k projections: spread over the previous
                    # pair's steps, one window every 5 steps, starting after
                    # pair 0's own windows are all emitted (step 16) so the
                    # weight-slot reuse can never cycle with the PE stream.
                    for p in range(1, GC):
                        base = (p - 1) * 4 * LC + 16
                        for j in range(8):      # 4 q windows then 4 k windows
                            qi, w = (0, j) if j < 4 else (1, j - 4)
                            extra[base + 5 * j].append(("w", p, qi, w))
                        extra[base].insert(0, ("wdma", p))

                    # prologue: weight DMAs + just enough projection to start
                    proj_w_dma(0, 0)
                    proj_w_dma(0, 1)
                    proj_v_dma()
                    proj_window(0, 0, 0)
                    proj_window(0, 1, 0)
                    for lc in range(4):
                        proj_v_unit(lc)
                    # remaining pair-0 windows, emitted before the steps that
                    # need them (k window j -> step 4j; q window w -> step 16w)
                    extra[1].append(("w", 0, 1, 1))
                    extra[3].append(("w", 0, 0, 1))
                    extra[5].append(("w", 0, 1, 2))
                    extra[9].append(("w", 0, 1, 3))
                    extra[11].append(("w", 0, 0, 2))
                    extra[13].append(("w", 0, 0, 3))

                    def emit_extra(u):
                        if u[0] == "v":
                            proj_v_unit(u[1])
                        elif u[0] == "w":
                            proj_window(u[1], u[2], u[3])
                        elif u[0] == "wdma":
                            proj_w_dma(u[1], 0)
                            proj_w_dma(u[1], 1)

                    sABs = {}
                    for s_ in range(LOOKAHEAD):
                        sABs[s_] = scores(*step_pqc(s_))
                    cA = cB = None
                    for s_ in range(NSTEP):
                        p, q, c = step_pqc(s_)
                        for u in extra[s_]:
                            emit_extra(u)
                        if c == 0:
                            cA = psBC.tile([65, 512], F32, tag="ctxA",
                                           name=f"cA{p}_{q}")
                            cB = psBC.tile([65, 512], F32, tag="ctxB",
                                           name=f"cB{p}_{q}")
                        if s_ + LOOKAHEAD < NSTEP:
                            sABs[s_ + LOOKAHEAD] = scores(*step_pqc(s_ + LOOKAHEAD))
                        sAB = sABs.pop(s_)
                        pt = ptp.tile([128, 1024], BF16, tag="pt")
                        nc.scalar.activation(
                            pt[:],
                            sAB[:],
                            mybir.ActivationFunctionType.Exp,
                            bias=mk_sb[:, c:c + 1],
                            scale=SCALE,
                        )
                        hA = 2 * p * HS
                        hB = (2 * p + 1) * HS
                        nc.tensor.matmul(
                            cA[:],
                            vt[c][:, hA:hA + HS],
                            pt[:, 0:512],
                            start=(c == 0), stop=(c == LC - 1),
                        )
                        nc.tensor.matmul(
                            cB[:],
                            vt[c][:, hB:hB + HS],
                            pt[:, 512:1024],
                            start=(c == 0), stop=(c == LC - 1),
                        )
                        if c == LC - 1:
                            lq = q * 512
                            nc.vector.tensor_copy(
                                ctxT[p][0:64, lq:lq + 512], cA[0:64, :]
                            )
                            nc.vector.tensor_copy(
                                ctxT[p][64:128, lq:lq + 512], cB[0:64, :]
                            )
                            nc.vector.tensor_copy(
                                den_a[32 * p:32 * p + 1, lq:lq + 512],
                                cA[64:65, :],
                            )
                            nc.vector.tensor_copy(
                                den_b[32 * p:32 * p + 1, lq:lq + 512],
                                cB[64:65, :],
                            )
                            if q == 3:
                                nc.sync.dma_start(
                                    out=out_d[p * 128:(p + 1) * 128, :],
                                    in_=ctxT[p][:],
                                )
                    # one strided DMA per den tile: partitions {0,32,64,96}
                    # -> den_d rows {0,2,4,6} (head A) / {1,3,5,7} (head B)
                    den_out = den_d.rearrange("(a b) l -> a b l", b=2)
                    nc.sync.dma_start(
                        out=den_out[:, 0, :],
                        in_=den_a.rearrange("(a b) l -> a b l", b=32)[:, 0, :],
                    )
                    nc.sync.dma_start(
                        out=den_out[:, 1, :],
                        in_=den_b.rearrange("(a b) l -> a b l", b=32)[:, 0, :],
                    )

            if hw_loop:
                with tc.For_i(0, hw_loop, 1):
                    body()
            else:
                for _rep in range(repeat):
                    body()

    _split_excess_waits(nc)
    return nc


def _prep_in_maps(x, attn_mask, cross_cls_sent, Wq, bq, Wk, bk, Wv, bv):
    bf = ml_dtypes.bfloat16
    x = np.asarray(x, dtype=np.float32)
    attn_mask = np.asarray(attn_mask, dtype=np.float32)
    cross = np.asarray(cross_cls_sent, dtype=np.float32)
    Wq = np.asarray(Wq, dtype=np.float32)
    bq = np.asarray(bq, dtype=np.float32)
    Wk = np.asarray(Wk, dtype=np.float32)
    bk = np.asarray(bk, dtype=np.float32)
    Wv = np.asarray(Wv, dtype=np.float32)
    bv = np.asarray(bv, dtype=np.float32)

    # packed xT: row p holds all 8 d-chunks of partition p back to back,
    # so the device loads x with a single [128, 8*2048] DMA.
    xt_bf = [
        np.ascontiguousarray(
            x[b].T.astype(bf).reshape(DC, 128, L).transpose(1, 0, 2)
            .reshape(128, DC * L)
        )
        for b in range(B)
    ]
    w_bf = {}
    for g in range(2):
        cols = slice(g * DG, (g + 1) * DG)
        w_bf[g] = (
            np.ascontiguousarray(Wq[:, cols].astype(bf)),
            np.ascontiguousarray(Wk[:, cols].astype(bf)),
            np.ascontiguousarray(Wv[:, cols].astype(bf)),
        )

    in_maps = []
    for c in range(NCORES):
        b = c // 2
        g = c % 2
        cols = slice(g * DG, (g + 1) * DG)
        qcross = cross[b] @ Wq[:, cols] + bq[cols]  # (512,) host matvec
        in_maps.append(
            {
                "xt": xt_bf[b],
                "wq": w_bf[g][0],
                "wk": w_bf[g][1],
                "wv": w_bf[g][2],
                "qcross": np.ascontiguousarray(
                    qcross.reshape(GC, 128).T.astype(np.float32)
                ),
                "bq": np.ascontiguousarray(bq[cols].reshape(GC, 128).T),
                "bk": np.ascontiguousarray(bk[cols].reshape(GC, 128).T),
                "bv": np.ascontiguousarray(bv[cols].reshape(1, DG)),
                "maskm": np.ascontiguousarray(
                    attn_mask[b, 0, 0].reshape(LC, 128).T
                ),
            }
        )
    return in_maps


def kernel(x, attn_mask, cross_cls_sent, Wq, bq, Wk, bk, Wv, bv):
    from concourse.bass_utils import run_bass_kernel_spmd

    if "nc" not in _CACHED:
        _CACHED["nc"] = _build_program()
    nc = _CACHED["nc"]

    in_maps = _prep_in_maps(
        x, attn_mask, cross_cls_sent, Wq, bq, Wk, bk, Wv, bv
    )
    res = run_bass_kernel_spmd(nc, in_maps, list(range(NCORES)))
    out = np.empty((B, L, D), dtype=np.float32)
    for c in range(NCORES):
        b = c // 2
        g = c % 2
        ct = res.results[c]["out"].astype(np.float32)     # [DG, L] unnormalized
        dn = res.results[c]["den"].astype(np.float32)     # [8, L]
        out[b][:, g * DG:(g + 1) * DG] = (ct / np.repeat(dn, HD, axis=0)).T
    return out


# revision 29
# speedup vs baseline: 1.2412x; 1.1400x over previous
"""Trainium2 Bass kernel for nn_CustomSelfAttention_24257975288159.

Reference computation (B=4, L=2048, D=1024, H=16, HD=64, fp32):
  q = x @ Wq + bq ; q[:, 1, :] = cross_cls_sent @ Wq + bq
  k = x @ Wk + bk ; v = x @ Wv + bv
  out = softmax(q k^T / sqrt(HD) + mask) v       (per head)

Sharding: 8 cores = batch (4) x head-group (2). Core c handles batch
c//2 and heads 8*(c%2)..8*(c%2)+7, i.e. columns 512*(c%2)..+512 of the
model dim; QKV weights are column-sharded per head group.

All matmuls run in bf16 (inputs rounded on host / by DVE) with fp32
PSUM accumulation; measured end-to-end rel err ~2e-3 vs the fp32
reference (gate is 2e-2). fp32r was 4x slower on HW (quarter-rate PE).

Division of labor vs the old fp32r kernel:
  - Host pre-transposes x and casts x/W to bf16, so the device loads
    xT [d-part, l-free] straight from HBM (no PE transpose phase).
  - Device emits unnormalized ctx^T [dh-part, lq-free] (fp16) plus
    per-head softmax denominators (fp16); host divides and transposes.
    This removes the on-device normalize/transpose epilogue.

Per-core device algorithm:
  1. Projections: qT,kT in [head-dim-part, l-free] layout (W chunk
     stationary, xT moving); v in [l-part, dh-free] layout with a ones
     column appended per head so the ctx matmul also produces softmax
     denominators (M=65).
  2. Attention per head-pair (p), lq quarter (q), lk chunk (c):
     transposed scores sT[lk-part, lq-free] via a row-tiled matmul pair
     (K=64 in each half of the PE array, both heads into one [128,1024]
     psum tile), one fused exp(SCALE*s + mask) on ScalarE covering both
     heads -> pt (bf16), then two M=65 ctx matmuls accumulating ctx^T
     plus the denominator row in PSUM.
  3. The exp stream on ScalarE is the bottleneck (~1.15us per step,
     256 steps); scores run LOOKAHEAD steps ahead and projection
     windows for the next pair are emitted one-per-few-steps between
     attention steps so ScalarE never waits at pair boundaries.
"""
import numpy as np
import ml_dtypes

import concourse.bass as bass
import concourse.mybir as mybir
import concourse.tile as tile

F32 = mybir.dt.float32
BF16 = mybir.dt.bfloat16
F16 = mybir.dt.float16

B, L, D, H = 4, 2048, 1024, 16
HD = D // H          # 64
SCALE = float(1.0 / np.sqrt(HD))
DG = D // 2          # 512 output columns per core (8 heads)
NCORES = 8
LC = L // 128        # 16 lk-chunks
DC = D // 128        # 8 d-chunks
GC = DG // 128       # 4 dh-chunks per core = head pairs
HS = HD + 1          # 65: v columns per head incl ones column

_CACHED = {}


# ---------------------------------------------------------------------------
# walrus workaround: this build rejects >1 sync-wait per instruction.
# Spill excess waits onto single-wait NOPs on the same engine.
# ---------------------------------------------------------------------------
def _split_excess_waits(nc, max_waits=1):
    counter = 0
    for fn in nc.m.functions:
        for blk in fn.blocks:
            il = blk.instructions
            out = []
            changed = False
            for ins in il:
                si = getattr(ins, "sync_info", None)
                waits = list(si.on_wait) if si is not None and si.on_wait else []
                if len(waits) > max_waits:
                    si.on_wait = waits[:max_waits]
                    for w in waits[max_waits:]:
                        counter += 1
                        out.append(
                            mybir.InstNoOp(
                                name=f"waitsplit_{counter}",
                                engine=ins.engine,
                                bass_nofuse=True,
                                sync_info=mybir.SyncInfo(on_wait=[w], on_update=[]),
                            )
                        )
                    changed = True
                out.append(ins)
            if changed:
                il.clear()
                il.extend(out)
    return counter


def _build_program(repeat=1, hw_loop=0):
    nc = bass.Bass()

    xt_d = nc.declare_dram_parameter("xt", [128, DC * L], BF16, isOutput=False)
    wq_d = nc.declare_dram_parameter("wq", [D, DG], BF16, isOutput=False)
    wk_d = nc.declare_dram_parameter("wk", [D, DG], BF16, isOutput=False)
    wv_d = nc.declare_dram_parameter("wv", [D, DG], BF16, isOutput=False)
    qc_d = nc.declare_dram_parameter("qcross", [128, GC], F32, isOutput=False)
    bq_d = nc.declare_dram_parameter("bq", [128, GC], F32, isOutput=False)
    bk_d = nc.declare_dram_parameter("bk", [128, GC], F32, isOutput=False)
    bv_d = nc.declare_dram_parameter("bv", [1, DG], F32, isOutput=False)
    mk_d = nc.declare_dram_parameter("maskm", [128, LC], F32, isOutput=False)
    out_d = nc.declare_dram_parameter("out", [DG, L], F16, isOutput=True)
    den_d = nc.declare_dram_parameter("den", [2 * GC, L], F16, isOutput=True)

    with tile.TileContext(nc, pool_alloc_mode="queue") as tc:
        with (
            tc.tile_pool(name="const", bufs=1) as const,
            tc.tile_pool(name="qkv", bufs=1) as qkv,
            tc.tile_pool(name="xt", bufs=1) as xtp,
            tc.tile_pool(name="wqk", bufs=1) as wqkp,
            tc.tile_pool(name="pt", bufs=3) as ptp,
            tc.tile_pool(name="fin", bufs=1) as finp,
        ):
            # ---- constants ----
            ones1 = const.tile([1, 128], F32)
            nc.vector.memset(ones1[:], 1.0)
            ones8 = const.tile([128, 8], BF16)
            nc.vector.memset(ones8[:], 1.0)
            qc_sb = const.tile([128, GC], F32)
            nc.sync.dma_start(out=qc_sb[:], in_=qc_d[:, :])
            bq_sb = const.tile([128, GC], F32)
            nc.sync.dma_start(out=bq_sb[:], in_=bq_d[:, :])
            bk_sb = const.tile([128, GC], F32)
            nc.sync.dma_start(out=bk_sb[:], in_=bk_d[:, :])
            bv_sb = const.tile([1, DG], F32)
            nc.sync.dma_start(out=bv_sb[:], in_=bv_d[:, :])
            mk_sb = const.tile([128, LC], F32)
            nc.sync.dma_start(out=mk_sb[:], in_=mk_d[:, :])
            bias_v = const.tile([128, DG], F32)

            def body():
                vt = [
                    qkv.tile([128, 8 * HS], BF16, tag=f"v{lc}", bufs=2,
                             name=f"v{lc}")
                    for lc in range(LC)
                ]
                xT_all = xtp.tile([128, DC * L], BF16, tag="xTall", bufs=2,
                                  name="xTall")
                xT = [xT_all[:, dc * L:(dc + 1) * L] for dc in range(DC)]
                ctxT = [
                    finp.tile([128, L], F16, tag=f"ctxT{p}", name=f"ctxT{p}")
                    for p in range(GC)
                ]
                # denominator rows: DVE output partition bases must be
                # 32-aligned, so pair p's two head rows live at partition
                # 32*p of two separate tiles (head A / head B). 128-partition
                # tiles so the final DMA can use a stride-32 rearrange view.
                den_a = finp.tile([128, L], F16, tag="denA", bufs=2)
                den_b = finp.tile([128, L], F16, tag="denB", bufs=2)

                # ---- load xT [d-part, l-free] straight from HBM ----
                # DMA instruction count is the scarce resource here: each
                # DMA costs ~2us of ring time regardless of size (HW), so
                # inputs are packed host-side and loaded with ONE DMA on the
                # ACT HWDGE ring. Outputs use the SP ring so neither ring
                # carries both directions (head-of-line blocking measured
                # +80us). SWDGE/gpsimd is unavailable (DynamicDMA disabled).
                nc.scalar.dma_start(out=xT_all[:], in_=xt_d[:, :])

                with tc.tile_pool(name="psBC", bufs=1, space="PSUM") as psBC:
                    with nc.named_scope("biasv"):
                        psb = psBC.tile([128, 512], F32, tag="proj", bufs=2)
                        nc.tensor.matmul(
                            psb[:], ones1[:], bv_sb[:], start=True, stop=True
                        )
                        nc.vector.tensor_copy(bias_v[:], psb[:])

                    # ---- projection work units ----
                    qk = [[None, None] for _ in range(GC)]  # [p][0]=qT, [1]=kT
                    wqk_sb = {}

                    def proj_w_dma(p, qi):
                        # per-chunk 2D DMAs: a [128,128] slice encodes as a
                        # cheap uniformly-strided descriptor pattern; a 3D
                        # batched variant (256B runs) measured much slower.
                        wd = wq_d if qi == 0 else wk_d
                        wt = wqkp.tile(
                            [128, DC * 128], BF16, tag="wqk", bufs=4,
                            name=f"w_{p}_{qi}",
                        )
                        for dc in range(DC):
                            nc.scalar.dma_start(
                                out=wt[:, dc * 128:(dc + 1) * 128],
                                in_=wd[dc * 128:(dc + 1) * 128,
                                       p * 128:(p + 1) * 128],
                            )
                        wqk_sb[(p, qi)] = [
                            wt[:, dc * 128:(dc + 1) * 128] for dc in range(DC)
                        ]

                    def proj_window(p, qi, w):
                        # one lq window (512) of the qT/kT projection for pair p
                        tagname = "qTs" if qi == 0 else "kTs"
                        if w == 0:
                            qk[p][qi] = qkv.tile(
                                [128, L], BF16, tag=tagname, bufs=2,
                                name=f"{tagname}{p}",
                            )
                        dst = qk[p][qi]
                        bias_sb = bq_sb if qi == 0 else bk_sb
                        wts = wqk_sb[(p, qi)]
                        with nc.named_scope(f"proj{p}_{qi}_{w}"):
                            psw = psBC.tile([128, 512], F32, tag="proj", bufs=2)
                            for dc in range(DC):
                                nc.tensor.matmul(
                                    psw[:],
                                    wts[dc],
                                    xT[dc][:, w * 512:(w + 1) * 512],
                                    start=(dc == 0),
                                    stop=(dc == DC - 1),
                                )
                            nc.vector.tensor_scalar_add(
                                dst[:, w * 512:(w + 1) * 512],
                                psw[:],
                                bias_sb[:, p:p + 1],
                            )
                            if qi == 0 and w == 0:
                                # q row-1 fix (CLS_sent cross query)
                                nc.vector.tensor_copy(
                                    dst[:, 1:2], qc_sb[:, p:p + 1]
                                )

                    wv_sb = []

                    def proj_v_dma():
                        wt = wqkp.tile(
                            [128, DC * DG], BF16, tag="wv", bufs=2, name="wv",
                        )
                        for dc in range(DC):
                            nc.scalar.dma_start(
                                out=wt[:, dc * DG:(dc + 1) * DG],
                                in_=wv_d[dc * 128:(dc + 1) * 128, :],
                            )
                            wv_sb.append(wt[:, dc * DG:(dc + 1) * DG])

                    def proj_v_unit(lc):
                        # v projection for one lk chunk, with ones columns
                        with nc.named_scope(f"projv{lc}"):
                            psv = psBC.tile([128, 512], F32, tag="proj", bufs=2)
                            for dc in range(DC):
                                nc.tensor.matmul(
                                    psv[:],
                                    xT[dc][:, lc * 128:(lc + 1) * 128],
                                    wv_sb[dc],
                                    start=(dc == 0),
                                    stop=(dc == DC - 1),
                                )
                            v = vt[lc]
                            ones_cols = v.rearrange(
                                "p (h s) -> p h s", s=HS
                            )[:, :, HD]
                            nc.vector.tensor_copy(ones_cols, ones8[:])
                            for h in range(8):
                                nc.vector.tensor_add(
                                    v[:, h * HS:h * HS + HD],
                                    psv[:, h * HD:(h + 1) * HD],
                                    bias_v[:, h * HD:(h + 1) * HD],
                                )

                    def scores(p, q, c):
                        q_t, k_t = qk[p][0], qk[p][1]
                        lq = q * 512
                        sAB = psBC.tile(
                            [128, 1024], F32, tag="sAB", bufs=2,
                            name=f"sAB_{p}_{q}_{c}",
                        )
                        nc.tensor.matmul(
                            sAB[:, 0:512],
                            k_t[0:64, c * 128:(c + 1) * 128],
                            q_t[0:64, lq:lq + 512],
                            start=True, stop=True,
                            tile_position=(0, 0),
                        )
                        nc.tensor.matmul(
                            sAB[:, 512:1024],
                            k_t[64:128, c * 128:(c + 1) * 128],
                            q_t[64:128, lq:lq + 512],
                            start=True, stop=True,
                            tile_position=(64, 0),
                        )
                        return sAB

                    # ---- emission schedule ----
                    # step s = (pair p, lq quarter q, lk chunk c)
                    NSTEP = GC * 4 * LC
                    LOOKAHEAD = 2

                    def step_pqc(s_):
                        p_, r = divmod(s_, 4 * LC)
                        q_, c_ = divmod(r, LC)
                        return p_, q_, c_

                    # Extra (non-attention) work units, emitted between steps
                    # so the PE interleaves them without starving ScalarE.
                    # extra[s] runs just before scores(s+LOOKAHEAD)/exp(s).
                    extra = [[] for _ in range(NSTEP)]
                    # v units: needed by ctx at step c (pair 0); keep 3 ahead.
                    for lc in range(4, LC):
                        extra[lc - 3].append(("v", lc))
                    # pair 1..3 q/k projections: spread over the previous
                    # pair's steps, one window every 5 steps, starting after
                    # pair 0's own windows are all emitted (step 16) so the
                    # weight-slot reuse can never cycle with the PE stream.
                    for p in range(1, GC):
                        base = (p - 1) * 4 * LC + 16
                        for j in range(8):      # 4 q windows then 4 k windows
                            qi, w = (0, j) if j < 4 else (1, j - 4)
                            extra[base + 5 * j].append(("w", p, qi, w))
                        extra[base].insert(0, ("wdma", p))

                    # prologue: weight DMAs + just enough projection to start
                    proj_w_dma(0, 0)
                    proj_w_dma(0, 1)
                    proj_v_dma()
                    proj_window(0, 0, 0)
                    proj_window(0, 1, 0)
                    for lc in range(4):
                        proj_v_unit(lc)
                    # remaining pair-0 windows, emitted before the steps that
                    # need them (k window j -> step 4j; q window w -> step 16w)
                    extra[1].append(("w", 0, 1, 1))
                    extra[3].append(("w", 0, 0, 1))
                    extra[5].append(("w", 0, 1, 2))
                    extra[9].append(("w", 0, 1, 3))
                    extra[11].append(("w", 0, 0, 2))
                    extra[13].append(("w", 0, 0, 3))

                    def emit_extra(u):
                        if u[0] == "v":
                            proj_v_unit(u[1])
                        elif u[0] == "w":
                            proj_window(u[1], u[2], u[3])
                        elif u[0] == "wdma":
                            proj_w_dma(u[1], 0)
                            proj_w_dma(u[1], 1)

                    sABs = {}
                    for s_ in range(LOOKAHEAD):
                        sABs[s_] = scores(*step_pqc(s_))
                    cA = cB = None
                    for s_ in range(NSTEP):
                        p, q, c = step_pqc(s_)
                        for u in extra[s_]:
                            emit_extra(u)
                        if c == 0:
                            cA = psBC.tile([65, 512], F32, tag="ctxA",
                                           name=f"cA{p}_{q}")
                            cB = psBC.tile([65, 512], F32, tag="ctxB",
                                           name=f"cB{p}_{q}")
                        if s_ + LOOKAHEAD < NSTEP:
                            sABs[s_ + LOOKAHEAD] = scores(*step_pqc(s_ + LOOKAHEAD))
                        sAB = sABs.pop(s_)
                        pt = ptp.tile([128, 1024], BF16, tag="pt")
                        nc.scalar.activation(
                            pt[:],
                            sAB[:],
                            mybir.ActivationFunctionType.Exp,
                            bias=mk_sb[:, c:c + 1],
                            scale=SCALE,
                        )
                        hA = 2 * p * HS
                        hB = (2 * p + 1) * HS
                        nc.tensor.matmul(
                            cA[:],
                            vt[c][:, hA:hA + HS],
                            pt[:, 0:512],
                            start=(c == 0), stop=(c == LC - 1),
                        )
                        nc.tensor.matmul(
                            cB[:],
                            vt[c][:, hB:hB + HS],
                            pt[:, 512:1024],
                            start=(c == 0), stop=(c == LC - 1),
                        )
                        if c == LC - 1:
                            lq = q * 512
                            nc.vector.tensor_copy(
                                ctxT[p][0:64, lq:lq + 512], cA[0:64, :]
                            )
                            nc.vector.tensor_copy(
                                ctxT[p][64:128, lq:lq + 512], cB[0:64, :]
                            )
                            nc.vector.tensor_copy(
                                den_a[32 * p:32 * p + 1, lq:lq + 512],
                                cA[64:65, :],
                            )
                            nc.vector.tensor_copy(
                                den_b[32 * p:32 * p + 1, lq:lq + 512],
                                cB[64:65, :],
                            )
                            if q == 3:
                                nc.sync.dma_start(
                                    out=out_d[p * 128:(p + 1) * 128, :],
                                    in_=ctxT[p][:],
                                )
                    # one strided DMA per den tile: partitions {0,32,64,96}
                    # -> den_d rows {0,2,4,6} (head A) / {1,3,5,7} (head B)
                    den_out = den_d.rearrange("(a b) l -> a b l", b=2)
                    nc.sync.dma_start(
                        out=den_out[:, 0, :],
                        in_=den_a.rearrange("(a b) l -> a b l", b=32)[:, 0, :],
                    )
                    nc.sync.dma_start(
                        out=den_out[:, 1, :],
                        in_=den_b.rearrange("(a b) l -> a b l", b=32)[:, 0, :],
                    )

            if hw_loop:
                with tc.For_i(0, hw_loop, 1):
                    body()
            else:
                for _rep in range(repeat):
                    body()

    _split_excess_waits(nc)
    return nc


def _prep_in_maps(x, attn_mask, cross_cls_sent, Wq, bq, Wk, bk, Wv, bv):
    bf = ml_dtypes.bfloat16
    x = np.asarray(x, dtype=np.float32)
    attn_mask = np.asarray(attn_mask, dtype=np.float32)
    cross = np.asarray(cross_cls_sent, dtype=np.float32)
    Wq = np.asarray(Wq, dtype=np.float32)
    bq = np.asarray(bq, dtype=np.float32)
    Wk = np.asarray(Wk, dtype=np.float32)
    bk = np.asarray(bk, dtype=np.float32)
    Wv = np.asarray(Wv, dtype=np.float32)
    bv = np.asarray(bv, dtype=np.float32)

    # packed xT: row p holds all 8 d-chunks of partition p back to back,
    # so the device loads x with a single [128, 8*2048] DMA.
    xt_bf = [
        np.ascontiguousarray(
            x[b].T.astype(bf).reshape(DC, 128, L).transpose(1, 0, 2)
            .reshape(128, DC * L)
        )
        for b in range(B)
    ]
    w_bf = {}
    for g in range(2):
        cols = slice(g * DG, (g + 1) * DG)
        w_bf[g] = (
            np.ascontiguousarray(Wq[:, cols].astype(bf)),
            np.ascontiguousarray(Wk[:, cols].astype(bf)),
            np.ascontiguousarray(Wv[:, cols].astype(bf)),
        )

    in_maps = []
    for c in range(NCORES):
        b = c // 2
        g = c % 2
        cols = slice(g * DG, (g + 1) * DG)
        qcross = cross[b] @ Wq[:, cols] + bq[cols]  # (512,) host matvec
        in_maps.append(
            {
                "xt": xt_bf[b],
                "wq": w_bf[g][0],
                "wk": w_bf[g][1],
                "wv": w_bf[g][2],
                "qcross": np.ascontiguousarray(
                    qcross.reshape(GC, 128).T.astype(np.float32)
                ),
                "bq": np.ascontiguousarray(bq[cols].reshape(GC, 128).T),
                "bk": np.ascontiguousarray(bk[cols].reshape(GC, 128).T),
                "bv": np.ascontiguousarray(bv[cols].reshape(1, DG)),
                "maskm": np.ascontiguousarray(
                    attn_mask[b, 0, 0].reshape(LC, 128).T
                ),
            }
        )
    return in_maps


def kernel(x, attn_mask, cross_cls_sent, Wq, bq, Wk, bk, Wv, bv):
    from concourse.bass_utils import run_bass_kernel_spmd

    if "nc" not in _CACHED:
        _CACHED["nc"] = _build_program()
    nc = _CACHED["nc"]

    in_maps = _prep_in_maps(
        x, attn_mask, cross_cls_sent, Wq, bq, Wk, bk, Wv, bv
    )
    res = run_bass_kernel_spmd(nc, in_maps, list(range(NCORES)))
    out = np.empty((B, L, D), dtype=np.float32)
    for c in range(NCORES):
        b = c // 2
        g = c % 2
        ct = res.results[c]["out"].astype(np.float32)     # [DG, L] unnormalized
        dn = res.results[c]["den"].astype(np.float32)     # [8, L]
        out[b][:, g * DG:(g + 1) * DG] = (ct / np.repeat(dn, HD, axis=0)).T
    return out
